# revision 51
# baseline (speedup 1.0000x reference)
"""Bahdanau-attention kernel for Trainium2 (8 NeuronCores, data-parallel over batch).

Computation (per batch b):
    enc_proj = h_enc @ W1.T + b1          # (L, D)   -- the big matmul
    dec_proj = h_dec @ W2.T + b2          # (D,)
    h        = tanh(enc_proj + dec_proj)  # (L, D)
    scores   = h @ V (+ bv)               # (L,)  -- bv cancels in softmax, dropped
    attn     = softmax(scores)            # no-max softmax: |scores| <= ||V||_1 ~ 16, exp is safe
    ctx      = attn @ enc_proj            # (B, D)

Device layout: everything transposed ("T-space", e/d on partitions):
  - h_enc is pre-transposed and cast to fp16 on the host (pure data
    marshalling, same rounding as an on-device cast-DMA would give), so the
    device streams rhs tiles straight from HBM -- no SBUF-SBUF transpose
    and half the HBM read traffic. All host layouts are DMA-contiguous.
  - dec_proj (67 MFLOP, 0.05% of the 137 GFLOP total) is folded into the
    per-batch bias on the host: it would otherwise gate the kernel head on
    a 2MB W2 load + 64 tiny LDW-bound matmuls in front of the main stream.
  - enc_projT[e, l] accumulated in PSUM via lhsT=W1T tiles, rhs=hT tiles
  - tanh fused with (b1+dec_proj+b2) bias on ACT; exp fused with Z-sum on ACT
  - scores: V-weighted running sum on DVE (one fused scalar_tensor_tensor
    per chunk), then one replicated ones-matmul folds the 128 partitions --
    keeps the PE almost exclusively on the main matmul
  - ctx partials via one fused scalar_tensor_tensor (mult + row-sum accum)
    per chunk on DVE (tensor_tensor_reduce faults real HW, STT does not)
  - the fold/exp/ctx/finalize block of segment i is EMITTED inside segment
    i+1's c-loop (fold-MM after the c0 group, exp/ctx after c1's MMs) so
    neither the in-order PE nor the in-order ACT ever waits cross-engine
  - the very last segment's epilogue is pipelined per 512-column group
    (split tanh/evac/V-sum for its final chunk) to shorten the kernel tail
  - divide by Z only at the very end (softmax normalizer cancels until then)
"""

import numpy as np

B, L, D = 32, 2048, 1024
NCORES = 8
NB = B // NCORES  # batches per core
P = 128
NCH = D // P      # 8 chunks of the d/e dimension
NH = 2            # l-halves per batch
LH = L // NH      # 1024
MAXSEG = 4        # zsl/ctx_sl slots per batch (last batch uses 3: half + 2 tail groups)

_cache = {}


def _build(reps=1):
    import concourse.bass as bass
    import concourse.tile as tile
    from concourse import bacc, mybir
    from concourse.bass import ts, ds
    from contextlib import ExitStack

    FP16 = mybir.dt.float16
    FP32 = mybir.dt.float32
    Alu = mybir.AluOpType
    Act = mybir.ActivationFunctionType
    X = mybir.AxisListType.X

    nc = bacc.Bacc("TRN2", name="bahdanau_attn")

    hT = nc.dram_tensor("hT", [NB, P, NCH, L], FP16, kind="ExternalInput")
    w1t = nc.dram_tensor("w1t", [P, NCH, D], FP16, kind="ExternalInput")   # [dpart, dchunk, e]
    ones_t = nc.dram_tensor("ones_t", [P, P], FP16, kind="ExternalInput")
    # b1 | V | bias(=dec_proj+b1+b2 per batch)
    f32s = nc.dram_tensor("f32s", [P, 2 * NCH + NCH * NB], FP32, kind="ExternalInput")
    out = nc.dram_tensor("ctx_out", [NB, P, NCH], FP32, kind="ExternalOutput")

    with tile.TileContext(nc) as tc, ExitStack() as ctx:
        wp = ctx.enter_context(tc.tile_pool(name="weights", bufs=1))
        tp = ctx.enter_context(tc.tile_pool(name="hT", bufs=4))
        ep = ctx.enter_context(tc.tile_pool(name="encproj", bufs=2))
        hp = ctx.enter_context(tc.tile_pool(name="htan", bufs=3))
        vp = ctx.enter_context(tc.tile_pool(name="vh", bufs=2))
        xp = ctx.enter_context(tc.tile_pool(name="exps", bufs=2))
        sp = ctx.enter_context(tc.tile_pool(name="scratch", bufs=3))
        fin = ctx.enter_context(tc.tile_pool(name="final", bufs=2))
        psA = ctx.enter_context(tc.tile_pool(name="psA", bufs=3, space="PSUM"))
        psS = ctx.enter_context(tc.tile_pool(name="psS", bufs=1, space="PSUM"))

        # ---- prologue: everything on the SP queue so the ACT queue never
        # runs DMA triggers; order = smalls, hT0 (the c0 rhs), then w1 split
        # per-chunk so the first c-loop's d-iterations pace with the
        # (serialized) DMA arrivals instead of waiting for the full 2MB
        f32_sb = wp.tile([P, 2 * NCH + NCH * NB], FP32)
        b1_sb = f32_sb[:, 0:NCH]
        v_sb = f32_sb[:, NCH : 2 * NCH]
        bias_sb = f32_sb[:, 2 * NCH :].rearrange("p (c b) -> p c b", b=NB)
        ones_sb = wp.tile([P, P], FP16)
        w1_sb = wp.tile([P, NCH, D], FP16)

        # PE warm-up: dummy matmuls on a never-written scratch tile -- no DMA
        # to wait for, so the PE is busy from ~1us and the HAM clock gate is
        # at full rate when the real matmul stream starts; the PSUM scratch
        # is overwritten (start=True) by the first fold later
        junk = wp.tile([P, P], FP16, name="junk")
        nc.vector.memset(junk, 0.0)
        ps_warm = psS.tile([P, LH], FP32, tag="sc", name="ps_warm")
        for _ in range(56):
            nc.tensor.matmul(
                ps_warm[:, :P], lhsT=junk, rhs=junk, start=True, stop=True
            )

        def load_w1():
            # tiny tensors slot between the hT0 half and w1 (~0.3us of the
            # serial DMA stream) so the first tanh's bias is ready in time
            nc.sync.dma_start(f32_sb, f32s[:])
            nc.sync.dma_start(ones_sb, ones_t[:])
            for d in range(NCH):
                nc.sync.dma_start(w1_sb[:, d, :], w1t[:, d, :])
            load_hT0_g1()

        # ---- software-pipelined main loop over l-segments ----
        segs = []
        for b in range(NB):
            segs += [(b, s, s * LH, LH, NH) for s in range(NH)]
        segs = segs * reps

        state = {}

        def emit_fold_mm(i):
            """Partition-fold of segment i's V-weighted partials (PE part)."""
            st = state[i]
            seglen = st["seglen"]
            ps_sc = psS.tile([P, LH], FP32, tag="sc", name="ps_sc")
            for g0 in range(0, seglen, LH // 2):
                gl = min(LH // 2, seglen - g0)
                nc.tensor.matmul(
                    ps_sc[:, ds(g0, gl)], lhsT=st["ones"],
                    rhs=st["vacc"][:, ds(g0, gl)],
                    start=True, stop=True,
                )
            st["ps_sc"] = ps_sc

        def emit_softmax_ctx(i, last=False):
            """exp + ctx partials (+ finalize) of segment i (ACT + DVE part)."""
            st = state.pop(i)
            b, s, l0, seglen, nseg = st["b"], st["s"], st["l0"], st["seglen"], st["nseg"]
            nc.scalar.activation(
                st["exp_rep"][:, ds(l0, seglen)], st["ps_sc"][:, :seglen], Act.Exp,
                accum_out=st["zsl"][:, s : s + 1],
            )
            # ctx partial: one fused scalar_tensor_tensor per chunk --
            # out = (enc * 1.0) * exp, accum_out = row-sum of out
            # (tensor_tensor_reduce faults real HW; STT+accum is the fast path)
            with nc.allow_low_precision("fp16 half-partials; |ctx_unnorm|<~1e3"):
                for c in range(NCH):
                    scratch = sp.tile([P, LH], FP16, tag="ttr")
                    nc.vector.scalar_tensor_tensor(
                        out=scratch[:, :seglen],
                        in0=st["enc_sb"][:, c, ds(l0, seglen)],
                        scalar=1.0,
                        in1=st["exp_rep"][:, ds(l0, seglen)],
                        op0=Alu.mult, op1=Alu.mult,
                        accum_out=st["ctx_sl"][:, c, s : s + 1],
                    )
            if s == nseg - 1:
                # finalize: ctx = ctx_unnorm / Z
                zsum = fin.tile([P, 1], FP32, tag="zsum")
                nc.vector.tensor_reduce(zsum, st["zsl"][:, :nseg], axis=X, op=Alu.add)
                recip = fin.tile([P, 1], FP32, tag="recip")
                nc.vector.reciprocal(recip, zsum)
                ctxf = fin.tile([P, NCH], FP32, tag="ctxf")
                ctxr = fin.tile([P, NCH], FP32, tag="ctxr")
                nc.vector.tensor_reduce(ctxr, st["ctx_sl"][:, :, :nseg], axis=X, op=Alu.add)
                nc.vector.tensor_scalar(
                    out=ctxf, in0=ctxr, scalar1=recip, scalar2=None, op0=Alu.mult
                )
                nc.scalar.dma_start(out[b], ctxf)

        batch_state = {}
        for i, (b, s, l0, seglen, nseg) in enumerate(segs):
            bb = b % NB
            if s == 0:
                batch_state = {
                    "exp_rep": xp.tile([P, L], FP16, tag="exp", name="exp_rep"),
                    "zsl": fin.tile([P, MAXSEG], FP32, tag="zsl", name="zsl"),
                    "ctx_sl": fin.tile([P, NCH, MAXSEG], FP32, tag="ctxsl", name="ctx_sl"),
                    "enc_sb": ep.tile([P, NCH, L], FP16, tag="enc", name="enc_sb"),
                }

            # pre-transposed fp16 tile straight from HBM: [dpart, dchunk, l]
            hT_sb = tp.tile([P, NCH, LH], FP16, tag="hT")
            if i == 0:
                # split the first load so the c0 g0 matmuls only gate on the
                # first half of the tile (and w1 streams between the halves)
                gl0 = seglen // 2
                nc.sync.dma_start(hT_sb[:, :, :gl0], hT[bb, :, :, ds(l0, gl0)])

                def load_hT0_g1(_t=hT_sb, _b=bb, _l0=l0, _sl=seglen, _g=gl0):
                    # per-d-chunk so the c0 g1 d-loop paces with arrivals
                    for d in range(NCH):
                        nc.sync.dma_start(
                            _t[:, d, _g:_sl], hT[_b, :, d, ds(_l0 + _g, _sl - _g)]
                        )

                load_w1()
            else:
                nc.sync.dma_start(hT_sb[:, :, :seglen], hT[bb, :, :, ds(l0, seglen)])

            # ping-pong pair for the V-weighted running sum (avoids in-place
            # DVE read/write of the same SBUF line, which faults real HW)
            vacc_a = vp.tile([P, LH], FP16, tag="vacc_a", name="vacc_a")
            vacc_b = vp.tile([P, LH], FP16, tag="vacc_b", name="vacc_b")
            vaccs = [vacc_a, vacc_b]
            enc_sb = batch_state["enc_sb"]
            for c in range(NCH):
                ps = psA.tile([P, LH], FP32, tag="mm")
                for g0 in range(0, seglen, LH // 2):  # stay within one PSUM bank
                    gl = min(LH // 2, seglen - g0)
                    for d in range(NCH):
                        nc.tensor.matmul(
                            ps[:, ds(g0, gl)],
                            lhsT=w1_sb[:, d, ts(c, P)],
                            rhs=hT_sb[:, d, ds(g0, gl)],
                            start=(d == 0),
                            stop=(d == NCH - 1),
                        )
                if i > 0:
                    if c == 0:
                        emit_fold_mm(i - 1)   # PE: right after the c0 group
                    elif c == 1:
                        emit_softmax_ctx(i - 1)  # ACT/DVE: after c0's tanh/evac
                # tanh + evac + V-weighted running sum; for the very last
                # chunk of the last segment these run per-512-group so the
                # tail's fold/exp/ctx can start half a chunk early
                tail_chunk = i == len(segs) - 1 and c == NCH - 1
                gsplits = (
                    [(0, seglen)] if not tail_chunk
                    else [(0, seglen // 2), (seglen // 2, seglen // 2)]
                )
                htan = hp.tile([P, LH], FP16, tag="htan")
                for q0, ql in gsplits:
                    # tanh(enc_projT + dec_proj + b1 + b2) on ACT, fused bias
                    nc.scalar.activation(
                        htan[:, ds(q0, ql)], ps[:, ds(q0, ql)], Act.Tanh,
                        bias=bias_sb[:, c, bb : bb + 1],
                    )
                    # evacuate enc_projT + b1 to fp16 SBUF (ACT only: keeps the
                    # PSUM-drain path off DVE, whose bursts would stall PE)
                    nc.scalar.activation(
                        enc_sb[:, c, ds(l0 + q0, ql)], ps[:, ds(q0, ql)],
                        Act.Identity, bias=b1_sb[:, c : c + 1],
                    )
                    with nc.allow_low_precision("fp16 V-partials; |vacc|<=0.25"):
                        if c == 0:
                            nc.vector.tensor_scalar(
                                out=vaccs[0][:, ds(q0, ql)], in0=htan[:, ds(q0, ql)],
                                scalar1=v_sb[:, 0:1], scalar2=None, op0=Alu.mult,
                            )
                        else:
                            nc.vector.scalar_tensor_tensor(
                                out=vaccs[c % 2][:, ds(q0, ql)],
                                in0=htan[:, ds(q0, ql)],
                                scalar=v_sb[:, c : c + 1],
                                in1=vaccs[(c - 1) % 2][:, ds(q0, ql)],
                                op0=Alu.mult, op1=Alu.add,
                            )

            st = dict(batch_state)
            st.update(
                {"b": bb, "s": s, "l0": l0, "seglen": seglen, "nseg": nseg,
                 "vacc": vaccs[(NCH - 1) % 2], "ones": ones_sb}
            )
            state[i] = st

        def emit_tail(i):
            """Last segment's epilogue, pipelined per 512-column group: fold,
            exp and ctx of group g overlap group g+1's chain, and the ctx
            reduces alternate between the (otherwise idle) ACT and DVE."""
            st = state.pop(i)
            b, s, l0 = st["b"], st["s"], st["l0"]
            GL = LH // 2
            ps_sc = psS.tile([P, LH], FP32, tag="sc", name="ps_sc")
            with nc.allow_low_precision("fp16 half-partials; |ctx_unnorm|<~1e3"):
                for g in range(2):
                    gs = ds(g * GL, GL)
                    al = ds(l0 + g * GL, GL)
                    nc.tensor.matmul(
                        ps_sc[:, gs], lhsT=st["ones"], rhs=st["vacc"][:, gs],
                        start=True, stop=True,
                    )
                    nc.scalar.activation(
                        st["exp_rep"][:, al], ps_sc[:, gs], Act.Exp,
                        accum_out=st["zsl"][:, s + g : s + g + 1],
                    )
                    if g == 1:
                        # Z and 1/Z compute during g1's ctx phase, off the
                        # end-of-kernel critical chain
                        zsum = fin.tile([P, 1], FP32, tag="zsum")
                        nc.vector.tensor_reduce(
                            zsum, st["zsl"][:, : s + 2], axis=X, op=Alu.add
                        )
                        recip = fin.tile([P, 1], FP32, tag="recip")
                        nc.vector.reciprocal(recip, zsum)
                        st["recip"] = recip
                    # split the serial epilogue between DVE (fused STT) and
                    # ACT (product on DVE, row-sum on ACT) so both engines
                    # share the tail
                    for c in range(NCH):
                        scratch = sp.tile([P, LH], FP16, tag="ttr")
                        if c % 3 == 1:
                            nc.vector.scalar_tensor_tensor(
                                out=scratch[:, :GL],
                                in0=st["enc_sb"][:, c, al],
                                scalar=1.0,
                                in1=st["exp_rep"][:, al],
                                op0=Alu.mult, op1=Alu.mult,
                                accum_out=st["ctx_sl"][:, c, s + g : s + g + 1],
                            )
                        else:
                            nc.vector.tensor_tensor(
                                scratch[:, :GL], st["enc_sb"][:, c, al],
                                st["exp_rep"][:, al], Alu.mult,
                            )
                            dummy = hp.tile([P, LH], FP16, tag="htan", name="dummy")
                            nc.scalar.activation(
                                dummy[:, :GL], scratch[:, :GL], Act.Identity,
                                accum_out=st["ctx_sl"][:, c, s + g : s + g + 1],
                            )
            nslots = s + 2
            ctxf = fin.tile([P, NCH], FP32, tag="ctxf")
            ctxr = fin.tile([P, NCH], FP32, tag="ctxr")
            nc.vector.tensor_reduce(ctxr, st["ctx_sl"][:, :, :nslots], axis=X, op=Alu.add)
            nc.vector.tensor_scalar(
                out=ctxf, in0=ctxr, scalar1=st["recip"], scalar2=None, op0=Alu.mult
            )
            nc.scalar.dma_start(out[b], ctxf)

        emit_tail(len(segs) - 1)

    nc.finalize()
    return nc


def _prep_shared(W1):
    # [dpart, dchunk, e] so the whole tensor is contiguous-descriptor DMAs
    return np.ascontiguousarray(
        W1.T.reshape(NCH, P, D).transpose(1, 0, 2).astype(np.float16)
    )


def kernel(h_enc, h_dec, W1, b1, W2, b2, V, bv):
    from concourse.bass_utils import run_bass_kernel_spmd

    h_enc = np.asarray(h_enc, dtype=np.float32)
    h_dec = np.asarray(h_dec, dtype=np.float32)
    W1 = np.asarray(W1, dtype=np.float32)
    b1 = np.asarray(b1, dtype=np.float32)
    W2 = np.asarray(W2, dtype=np.float32)
    b2 = np.asarray(b2, dtype=np.float32)
    V = np.asarray(V, dtype=np.float32)

    if "nc" not in _cache:
        _cache["nc"] = _build()
    nc = _cache["nc"]

    w1t = _prep_shared(W1)
    ones = np.ones((P, P), dtype=np.float16)

    b1t = b1.reshape(NCH, P).T.astype(np.float32)          # [P, NCH]
    vt = V.reshape(NCH, P).T.astype(np.float32)            # [P, NCH]
    # per-batch tanh bias: dec_proj + b1 + b2 (67 MFLOP on host)
    bias_full = h_dec @ W2.T + (b1 + b2)[None, :]          # [B, D] fp32

    # host-side marshalling: cast once (contiguous, fast), then per-core
    # T-space relayout hT[b, dpart, dchunk, l] = h_enc[b, l, dchunk*P+dpart]
    h16 = h_enc.astype(np.float16)

    in_maps = []
    for core in range(NCORES):
        sl = slice(core * NB, (core + 1) * NB)
        # bias[p, c, b] = bias_full[b, c*128+p] flattened to [P, NCH*NB]
        bias_c = bias_full[sl].T.reshape(NCH, P, NB).transpose(1, 0, 2).reshape(P, NCH * NB)
        f32s = np.ascontiguousarray(
            np.concatenate([b1t, vt, bias_c], axis=1).astype(np.float32)
        )
        hT_core = np.ascontiguousarray(
            h16[sl].reshape(NB, L, NCH, P).transpose(0, 3, 2, 1)
        )
        in_maps.append(
            {
                "hT": hT_core,
                "w1t": w1t,
                "ones_t": ones,
                "f32s": f32s,
            }
        )

    try:
        res = run_bass_kernel_spmd(nc, in_maps, core_ids=list(range(NCORES)))
    except Exception:
        # transient device-state faults have been observed right after a
        # previous process crashed the core; one retry clears them
        import time

        time.sleep(10)
        res = run_bass_kernel_spmd(nc, in_maps, core_ids=list(range(NCORES)))
    outs = []
    for core in range(NCORES):
        o = res.results[core]["ctx_out"]  # [NB, P, NCH]
        outs.append(o.transpose(0, 2, 1).reshape(NB, D))  # e = c*128 + p
    return np.concatenate(outs, axis=0).astype(np.float32)
